# revision 39
# baseline (speedup 1.0000x reference)
"""Trainium2 Bass kernel for nn_CliffordInteractionExpert (v2: bf16 blade-major).

Math (CliffordAlgebra p=3,q=1: ALG=16 blades, D=1024 = 64 chunks of 16):
  All three shifts are linear, so they collapse into one stencil:
      u = 3x - x<<1 - x<<2 - x<<4   (roll along T, wraparound)
  out = x + gate * [ sb*(x_p u_q - x_q u_p) at bivector blades,
                     ss*sum_d w0*x*u       at d=0 ]
  gate = sigmoid(x @ gate_w + gate_b)

Key layout decisions (vs v1):
  - Everything in bf16 on device (tolerance 2e-2; measured err stays ~4e-3).
  - Host permutes D into "blade-major" order: position p*64+n holds blade
    ORD[p] of chunk n, with ORD = [8,3,5,6,7,15, 0,1,2,4, 9,10,12, 11,13,14].
    * w0-negative blades occupy positions 0..5  -> the Cayley-diagonal
      reduction is two contiguous fused multiply-reduce ops (neg / pos).
    * vector blades 1,2,4 at positions 7,8,9 and 8 at position 0 -> the six
      wedge pair-products group into 6 strided step-1 ops (DVE 2x mode).
    * bivector outputs 3,5,6 at positions 1,2,3 and 9,10,12 at 10,11,12 ->
      gated output assembly is two step-1 ops per 128-row block.
  - Output is compact [T, 392] (384 bivector cols + 1 scalar col + pad):
    only 385 of 1024 positions differ from x; host scatters them into a
    copy of x. Cuts store traffic 5x.
  - Stencil u on TensorE as banded-matrix matmuls (bf16, half the cycles of
    fp32); halo rows for block j>0 come from partitions 124..127 of block
    j-1 in-tile; only j=0 reads a 4-row halo from DRAM.
  - PSUM is split in 4 quarters (2 blocks each) with bufs=2 so evacuation
    (ScalarE copy -> bf16 SBUF) overlaps the next quarter's matmuls.
"""

import numpy as np
import ml_dtypes

BF16 = ml_dtypes.bfloat16
ALG = 16
SHIFTS = (1, 2, 4)
# blade at position p of each 64-wide block (see module docstring)
ORD = [8, 3, 5, 6, 7, 15, 0, 1, 2, 4, 9, 10, 12, 11, 13, 14]
# wedge pairs (p_blade, q_blade) -> bivector k = p^q; order chosen so
# k-positions are [1,2,3, 10,11,12] in ORD space (step-1 groups)
PAIRS = [(1, 2), (1, 4), (2, 4), (1, 8), (2, 8), (4, 8)]  # k = 3,5,6, 9,10,12
POS = {b: p for p, b in enumerate(ORD)}

_PROG_CACHE: dict = {}
TRACE = False
LAST_RESULT = None


def _sigmoid_f32(v: float) -> float:
    return float(1.0 / (1.0 + np.exp(-np.float32(v), dtype=np.float32)))


def _stencil_weights():
    """u = 3x - x[t-1] - x[t-2] - x[t-4] as lhsT banded matrices.

    wm[s, t]: weight of in-block row s for output row t (128x128).
    ww[h, t]: weight of halo row h (the 4 rows preceding the block).
    """
    wm = np.zeros((128, 128), np.float32)
    ww = np.zeros((4, 128), np.float32)
    for t in range(128):
        wm[t, t] = 3.0
        for k in SHIFTS:
            if t - k >= 0:
                wm[t - k, t] -= 1.0
            else:
                ww[4 + t - k, t] -= 1.0
    return wm.astype(BF16), ww.astype(BF16)


def _subap(base, elem_off, dims):
    """AP at base's tensor with extra element offset and explicit free dims."""
    import concourse.bass as bass

    return bass.AP(tensor=base.tensor, offset=base.offset + elem_off,
                   ap=[list(base.ap[0])] + [list(d) for d in dims])


def build_program(T: int, D: int, ss: float, sb: float, gb: float):
    from contextlib import ExitStack

    import concourse.bacc as bacc
    import concourse.mybir as mybir
    from concourse.tile import TileContext

    bf16 = mybir.dt.bfloat16
    f32 = mybir.dt.float32
    J = 8                  # 128-row blocks per iteration
    ROWS = 128 * J         # 1024
    W = 392                # compact output row width
    assert T % ROWS == 0 and D == 1024
    n_iter = T // ROWS

    nc = bacc.Bacc("TRN2", target_bir_lowering=False, debug=False)
    x_d = nc.dram_tensor("x", [T, D], bf16, kind="ExternalInput")
    gw_d = nc.dram_tensor("gwcol", [128, D // 128], bf16, kind="ExternalInput")
    wm_d = nc.dram_tensor("wmain", [128, 128], bf16, kind="ExternalInput")
    ww_d = nc.dram_tensor("wwrap", [4, 128], bf16, kind="ExternalInput")
    out_d = nc.dram_tensor("out", [T, W], bf16, kind="ExternalOutput")

    mult = mybir.AluOpType.mult
    add = mybir.AluOpType.add
    sub_op = mybir.AluOpType.subtract

    with TileContext(nc) as tc, ExitStack() as ctx:
        consts = ctx.enter_context(tc.tile_pool(name="consts", bufs=1))
        xp = ctx.enter_context(tc.tile_pool(name="xp", bufs=2))
        xtp = ctx.enter_context(tc.tile_pool(name="xtp", bufs=2))
        utp = ctx.enter_context(tc.tile_pool(name="utp", bufs=2))
        wp = ctx.enter_context(tc.tile_pool(name="wp", bufs=2))
        outp = ctx.enter_context(tc.tile_pool(name="outp", bufs=2))
        scrp = ctx.enter_context(tc.tile_pool(name="scrp", bufs=2))
        smallp = ctx.enter_context(tc.tile_pool(name="smallp", bufs=4))
        halop = ctx.enter_context(tc.tile_pool(name="halop", bufs=2))
        psum = ctx.enter_context(tc.tile_pool(name="psum", bufs=2, space="PSUM"))

        gw_sb = consts.tile([128, D // 128], bf16)
        nc.sync.dma_start(out=gw_sb[:], in_=gw_d[:])
        wm_sb = consts.tile([128, 128], bf16)
        nc.sync.dma_start(out=wm_sb[:], in_=wm_d[:])
        ww_sb = consts.tile([4, 128], bf16)
        nc.sync.dma_start(out=ww_sb[:], in_=ww_d[:])
        id1 = consts.tile([1, 1], bf16)
        nc.vector.memset(id1[:], 1.0)

        def emit_loads(it):
            base = it * ROWS
            # x tile [128, (j, d)]: row t = base + 128j + p
            x_t = xp.tile([128, J * D], bf16, tag="x", bufs=3)
            nc.sync.dma_start(
                out=x_t[:].rearrange("p (j d) -> p j d", j=J),
                in_=x_d[base:base + ROWS, :].rearrange("(j p) d -> p j d", p=128),
            )
            # halo tile [4, (j, d)]: rows base+128j-4 .. base+128j
            halo_t = halop.tile([4, J * D], bf16, tag="halo", bufs=3)
            if it == 0:
                # j=0 wraps to the last 4 rows of the sequence
                nc.sync.dma_start(
                    out=halo_t[:].rearrange("p (j d) -> p j d", j=J)[:, 0, :],
                    in_=x_d[T - 4:T, :],
                )
                nc.sync.dma_start(
                    out=halo_t[:].rearrange("p (j d) -> p j d", j=J)[:, 1:, :],
                    in_=_subap(x_d[124:128, :], 0, [[128 * D, J - 1], [1, D]]),
                )
            else:
                nc.sync.dma_start(
                    out=halo_t[:].rearrange("p (j d) -> p j d", j=J),
                    in_=_subap(x_d[base - 4:base, :], 0, [[128 * D, J], [1, D]]),
                )
            return x_t, halo_t

        pending_store = None
        cur = emit_loads(0)
        for it in range(n_iter):
            base = it * ROWS
            x_t, halo_t = cur
            # prefetch next iteration's tiles BEFORE this iteration's
            # transpose occupies the sync queue, so the PE never starves
            if it + 1 < n_iter:
                cur = emit_loads(it + 1)

            # ---- d-major copy of x via the DMA xbar transpose engine:
            # xT[dlo, (j*8+dc), t] = x[t, j*1024 + dc*128 + dlo]
            xT_t = xtp.tile([128, J * D], bf16)
            nc.sync.dma_start_transpose(
                out=xT_t[:].rearrange("p (c q) -> p c q", q=128), in_=x_t[:])

            # previous iteration's store goes last on the queue
            if pending_store is not None:
                pending_store()
                pending_store = None

            # ---- stencil u on TensorE; 8 PSUM eighths, processed in pairs
            # with same-lhsT matmuls batched (runs of 4) so the PE weight
            # buffer isn't thrashed on every instruction ----
            u_t = utp.tile([128, J * D], bf16)
            for jp in range(J // 2):
                ups_pair = []
                for jj in range(2):
                    j = 2 * jp + jj
                    u_ps = psum.tile([128, D], f32, tag="ups", bufs=3)
                    ups_pair.append((j, u_ps))
                    for c in range(2):
                        nc.tensor.matmul(
                            u_ps[:, c * 512:(c + 1) * 512], lhsT=wm_sb[:],
                            rhs=x_t[:, j * D + c * 512:j * D + (c + 1) * 512],
                            start=True, stop=False)
                for j, u_ps in ups_pair:
                    for c in range(2):
                        nc.tensor.matmul(
                            u_ps[:, c * 512:(c + 1) * 512], lhsT=ww_sb[:],
                            rhs=halo_t[:, j * D + c * 512:j * D + (c + 1) * 512],
                            start=False, stop=True)
                for j, u_ps in ups_pair:
                    # evacuate -> bf16 SBUF (ScalarE), folding the Cayley
                    # signs: u_t holds uw = w0 * u (neg blades at cols 0..383)
                    nc.scalar.activation(
                        out=u_t[:, j * D:j * D + 384], in_=u_ps[:, :384],
                        func=mybir.ActivationFunctionType.Copy, scale=-1.0)
                    nc.scalar.activation(
                        out=u_t[:, j * D + 384:(j + 1) * D], in_=u_ps[:, 384:],
                        func=mybir.ActivationFunctionType.Copy)

            # ---- gate matvec on TensorE: lhsT = xT chunk, rhs = gw column --
            gps = psum.tile([128, J], f32, tag="gps", bufs=2)
            for j in range(J):
                for dc in range(8):
                    nc.tensor.matmul(
                        gps[:, j:j + 1],
                        lhsT=xT_t[:, (j * 8 + dc) * 128:(j * 8 + dc + 1) * 128],
                        rhs=gw_sb[:, dc:dc + 1],
                        start=(dc == 0), stop=(dc == 7))
            gate = smallp.tile([128, J], f32, tag="gate")
            nc.scalar.activation(out=gate[:], in_=gps[:],
                                 func=mybir.ActivationFunctionType.Sigmoid,
                                 bias=float(gb), scale=1.0)
            gate_sb = smallp.tile([128, J], f32, tag="gate_sb")
            nc.vector.tensor_scalar_mul(gate_sb[:], gate[:], float(sb))

            # ---- scalar part: sacc[:, j] = sum_d x*uw (signs pre-folded) --
            # fused multiply+reduce STT; 1x mode, but one pass per block
            sacc = smallp.tile([128, J], f32, tag="sacc")
            scr_s = scrp.tile([128, D], bf16, tag="scr_s", bufs=1)
            for j in range(J):
                nc.vector.scalar_tensor_tensor(
                    out=scr_s[:],
                    in0=x_t[:, j * D:(j + 1) * D], scalar=1.0,
                    in1=u_t[:, j * D:(j + 1) * D], op0=mult, op1=mult,
                    accum_out=sacc[:, j:j + 1],
                )

            # ---- wedge pair products (positions: 1->7, 2->8, 4->9, 8->0) --
            wF = wp.tile([128, 6 * J * 64], bf16, tag="wF", bufs=1)
            wR = wp.tile([128, 6 * J * 64], bf16, tag="wR", bufs=1)
            w_t = wp.tile([128, 6 * J * 64], bf16, tag="w", bufs=1)
            jn = [[D, J], [1, 64]]           # (j, n) dims on x/u tiles
            PJ = J * 64                       # pair stride on w tiles

            def prod(dst, pr0, npr, xoff, xstep, uoff, ustep):
                nc.vector.tensor_tensor(
                    out=_subap(dst[:], pr0 * PJ, [[PJ, npr], [64, J], [1, 64]]),
                    in0=_subap(x_t[:], xoff * 64, [[xstep * 64, npr]] + jn),
                    in1=_subap(u_t[:], uoff * 64, [[ustep * 64, npr]] + jn),
                    op=mult,
                )

            # forward x_p * uw_q: pairs (1,2),(1,4) | (2,4) | (1,8),(2,8),(4,8)
            # NOTE uw at position 0 is -u_8, so the pair-3..5 forward products
            # come out negated; those pairs use w = -(F' + R') handled below.
            prod(wF, 0, 2, 7, 0, 8, 1)
            prod(wF, 2, 1, 8, 1, 9, 0)
            prod(wF, 3, 3, 7, 1, 0, 0)
            # reverse x_q * uw_p (positions 7,8,9 are w0-positive: plain u)
            prod(wR, 0, 2, 8, 1, 7, 0)
            prod(wR, 2, 1, 9, 1, 8, 0)
            prod(wR, 3, 3, 0, 0, 7, 1)
            # pairs 0..2: w = F - R ; pairs 3..5: wB = F' + R' = -w
            nc.vector.tensor_tensor(out=w_t[:, :3 * PJ], in0=wF[:, :3 * PJ],
                                    in1=wR[:, :3 * PJ], op=sub_op)
            nc.vector.tensor_tensor(out=w_t[:, 3 * PJ:], in0=wF[:, 3 * PJ:],
                                    in1=wR[:, 3 * PJ:], op=add)

            # ---- scale by sb*gate (per-block TS, 4x), then assemble ----
            gwt = wp.tile([128, 6 * J * 64], bf16, tag="gwt", bufs=1)
            for j in range(J):
                nc.vector.tensor_scalar(
                    out=_subap(gwt[:], j * 64, [[PJ, 6], [1, 64]]),
                    in0=_subap(w_t[:], j * 64, [[PJ, 6], [1, 64]]),
                    scalar1=gate_sb[:, j:j + 1], scalar2=None, op0=mult,
                )
            out_t = outp.tile([128, J * W], bf16)
            # neg bivectors k=3,5,6 at positions 1,2,3 -> cols 0..191
            nc.vector.tensor_tensor(
                out=_subap(out_t[:], 0, [[W, J], [64, 3], [1, 64]]),
                in0=_subap(gwt[:], 0, [[64, J], [PJ, 3], [1, 64]]),
                in1=_subap(x_t[:], 64, [[D, J], [64, 3], [1, 64]]),
                op=add,
            )
            # pos bivectors k=9,10,12 at 10,11,12 -> cols 192..383 (w = -wB)
            nc.vector.tensor_tensor(
                out=_subap(out_t[:], 192, [[W, J], [64, 3], [1, 64]]),
                in0=_subap(x_t[:], 640, [[D, J], [64, 3], [1, 64]]),
                in1=_subap(gwt[:], 3 * PJ, [[64, J], [PJ, 3], [1, 64]]),
                op=sub_op,
            )

            # ---- scalar column: out[:, 384] = x0 + ss*gate*sacc ----
            gs = smallp.tile([128, J], f32, tag="gs")
            nc.vector.tensor_tensor(out=gs[:], in0=gate[:], in1=sacc[:], op=mult)
            nc.vector.scalar_tensor_tensor(
                out=_subap(out_t[:], 384, [[W, J]]),
                in0=gs[:], scalar=float(ss),
                in1=_subap(x_t[:], 384, [[D, J]]),   # blade 0 at position 6
                op0=mult, op1=add,
            )

            # ---- store compact tile (deferred; see top of loop) ----
            def make_store(base=base, out_t=out_t):
                def store():
                    nc.sync.dma_start(
                        out=out_d[base:base + ROWS, :].rearrange(
                            "(j p) w -> p j w", p=128),
                        in_=out_t[:].rearrange("p (j w) -> p j w", j=J),
                    )
                return store
            pending_store = make_store()

        pending_store()

    nc.compile()
    return nc


def _get_program(T, D, ss, sb, gb):
    key = (T, D, round(ss, 9), round(sb, 9), round(gb, 9))
    if key not in _PROG_CACHE:
        _PROG_CACHE[key] = build_program(T, D, ss, sb, gb)
    return _PROG_CACHE[key]


def _permute_cols(a2d, D):
    """[.., D] f32 -> blade-major bf16: position p*64+n <- blade ORD[p], chunk n."""
    n = D // ALG
    r = a2d.reshape(a2d.shape[:-1] + (n, ALG))
    r = r[..., ORD]                      # [..., n, 16] with blades reordered
    r = np.swapaxes(r, -1, -2)           # [..., 16, n]
    return np.ascontiguousarray(r.reshape(a2d.shape[:-1] + (D,)).astype(BF16))


def kernel(x, gate_w, gate_b, scalar_weight, bivector_weight):
    x = np.asarray(x, np.float32)
    B, T, D = x.shape
    assert B == 8 and D == 1024

    ss = _sigmoid_f32(np.asarray(scalar_weight).reshape(-1)[0])
    sb = _sigmoid_f32(np.asarray(bivector_weight).reshape(-1)[0])
    gb = float(np.asarray(gate_b).reshape(-1)[0])

    nc = _get_program(T, D, ss, sb, gb)

    from concourse.bass_utils import run_bass_kernel_spmd

    gw = np.asarray(gate_w, np.float32).reshape(D)
    gw_bm = _permute_cols(gw[None, :], D)[0].astype(np.float32)
    # d-major gw for the TensorE matvec: gwcol[dlo, dc] = gw_bm[dc*128+dlo]
    gwcol = np.ascontiguousarray(gw_bm.reshape(8, 128).T.astype(BF16))
    wm, ww = _stencil_weights()
    in_maps = []
    for c in range(B):
        in_maps.append({
            "x": _permute_cols(x[c], D),
            "gwcol": gwcol,
            "wmain": wm,
            "wwrap": ww,
        })
    res = run_bass_kernel_spmd(nc, in_maps, list(range(B)), trace=TRACE)
    global LAST_RESULT
    LAST_RESULT = res

    # host-side scatter: only 385 of 1024 positions differ from x
    out = x.copy()
    kcols = np.array([16 * n + (p ^ q) for (p, q) in PAIRS for n in range(64)])
    for c in range(B):
        o = np.asarray(res.results[c]["out"], dtype=np.float32)  # [T, 392]
        out[c][:, kcols] = o[:, :384]
        out[c][:, 0] = o[:, 384]
    return out


# revision 40
# speedup vs baseline: 1.0254x; 1.0254x over previous
"""Trainium2 Bass kernel for nn_CliffordInteractionExpert (v2: bf16 blade-major).

Math (CliffordAlgebra p=3,q=1: ALG=16 blades, D=1024 = 64 chunks of 16):
  All three shifts are linear, so they collapse into one stencil:
      u = 3x - x<<1 - x<<2 - x<<4   (roll along T, wraparound)
  out = x + gate * [ sb*(x_p u_q - x_q u_p) at bivector blades,
                     ss*sum_d w0*x*u       at d=0 ]
  gate = sigmoid(x @ gate_w + gate_b)

Key layout decisions (vs v1):
  - Everything in bf16 on device (tolerance 2e-2; measured err stays ~4e-3).
  - Host permutes D into "blade-major" order: position p*64+n holds blade
    ORD[p] of chunk n, with ORD = [8,3,5,6,7,15, 0,1,2,4, 9,10,12, 11,13,14].
    * w0-negative blades occupy positions 0..5  -> the Cayley-diagonal
      reduction is two contiguous fused multiply-reduce ops (neg / pos).
    * vector blades 1,2,4 at positions 7,8,9 and 8 at position 0 -> the six
      wedge pair-products group into 6 strided step-1 ops (DVE 2x mode).
    * bivector outputs 3,5,6 at positions 1,2,3 and 9,10,12 at 10,11,12 ->
      gated output assembly is two step-1 ops per 128-row block.
  - Output is compact [T, 392] (384 bivector cols + 1 scalar col + pad):
    only 385 of 1024 positions differ from x; host scatters them into a
    copy of x. Cuts store traffic 5x.
  - Stencil u on TensorE as banded-matrix matmuls (bf16, half the cycles of
    fp32); halo rows for block j>0 come from partitions 124..127 of block
    j-1 in-tile; only j=0 reads a 4-row halo from DRAM.
  - PSUM is split in 4 quarters (2 blocks each) with bufs=2 so evacuation
    (ScalarE copy -> bf16 SBUF) overlaps the next quarter's matmuls.
"""

import numpy as np
import ml_dtypes

BF16 = ml_dtypes.bfloat16
ALG = 16
SHIFTS = (1, 2, 4)
# blade at position p of each 64-wide block (see module docstring)
ORD = [8, 3, 5, 6, 7, 15, 0, 1, 2, 4, 9, 10, 12, 11, 13, 14]
# wedge pairs (p_blade, q_blade) -> bivector k = p^q; order chosen so
# k-positions are [1,2,3, 10,11,12] in ORD space (step-1 groups)
PAIRS = [(1, 2), (1, 4), (2, 4), (1, 8), (2, 8), (4, 8)]  # k = 3,5,6, 9,10,12
POS = {b: p for p, b in enumerate(ORD)}

_PROG_CACHE: dict = {}
TRACE = False
LAST_RESULT = None


def _sigmoid_f32(v: float) -> float:
    return float(1.0 / (1.0 + np.exp(-np.float32(v), dtype=np.float32)))


def _stencil_weights():
    """u = 3x - x[t-1] - x[t-2] - x[t-4] as lhsT banded matrices.

    wm[s, t]: weight of in-block row s for output row t (128x128).
    ww[h, t]: weight of halo row h (the 4 rows preceding the block).
    """
    wm = np.zeros((128, 128), np.float32)
    ww = np.zeros((4, 128), np.float32)
    for t in range(128):
        wm[t, t] = 3.0
        for k in SHIFTS:
            if t - k >= 0:
                wm[t - k, t] -= 1.0
            else:
                ww[4 + t - k, t] -= 1.0
    return wm.astype(BF16), ww.astype(BF16)


def _subap(base, elem_off, dims):
    """AP at base's tensor with extra element offset and explicit free dims."""
    import concourse.bass as bass

    return bass.AP(tensor=base.tensor, offset=base.offset + elem_off,
                   ap=[list(base.ap[0])] + [list(d) for d in dims])


def build_program(T: int, D: int, ss: float, sb: float, gb: float):
    from contextlib import ExitStack

    import concourse.bacc as bacc
    import concourse.mybir as mybir
    from concourse.tile import TileContext

    bf16 = mybir.dt.bfloat16
    f32 = mybir.dt.float32
    J = 8                  # 128-row blocks per iteration
    ROWS = 128 * J         # 1024
    W = 392                # compact output row width
    assert T % ROWS == 0 and D == 1024
    n_iter = T // ROWS

    nc = bacc.Bacc("TRN2", target_bir_lowering=False, debug=False)
    x_d = nc.dram_tensor("x", [T, D], bf16, kind="ExternalInput")
    gw_d = nc.dram_tensor("gwcol", [128, D // 128], bf16, kind="ExternalInput")
    wm_d = nc.dram_tensor("wmain", [128, 128], bf16, kind="ExternalInput")
    ww_d = nc.dram_tensor("wwrap", [4, 128], bf16, kind="ExternalInput")
    out_d = nc.dram_tensor("out", [T, W], bf16, kind="ExternalOutput")

    mult = mybir.AluOpType.mult
    add = mybir.AluOpType.add
    sub_op = mybir.AluOpType.subtract

    with TileContext(nc) as tc, ExitStack() as ctx:
        consts = ctx.enter_context(tc.tile_pool(name="consts", bufs=1))
        xp = ctx.enter_context(tc.tile_pool(name="xp", bufs=2))
        xtp = ctx.enter_context(tc.tile_pool(name="xtp", bufs=2))
        utp = ctx.enter_context(tc.tile_pool(name="utp", bufs=2))
        wp = ctx.enter_context(tc.tile_pool(name="wp", bufs=2))
        outp = ctx.enter_context(tc.tile_pool(name="outp", bufs=2))
        scrp = ctx.enter_context(tc.tile_pool(name="scrp", bufs=2))
        smallp = ctx.enter_context(tc.tile_pool(name="smallp", bufs=4))
        halop = ctx.enter_context(tc.tile_pool(name="halop", bufs=2))
        psum = ctx.enter_context(tc.tile_pool(name="psum", bufs=2, space="PSUM"))

        gw_sb = consts.tile([128, D // 128], bf16)
        nc.sync.dma_start(out=gw_sb[:], in_=gw_d[:])
        wm_sb = consts.tile([128, 128], bf16)
        nc.sync.dma_start(out=wm_sb[:], in_=wm_d[:])
        ww_sb = consts.tile([4, 128], bf16)
        nc.sync.dma_start(out=ww_sb[:], in_=ww_d[:])
        id1 = consts.tile([1, 1], bf16)
        nc.vector.memset(id1[:], 1.0)

        def emit_loads(it):
            base = it * ROWS
            # x tile [128, (j, d)]: row t = base + 128j + p
            x_t = xp.tile([128, J * D], bf16, tag="x", bufs=3)
            nc.sync.dma_start(
                out=x_t[:].rearrange("p (j d) -> p j d", j=J),
                in_=x_d[base:base + ROWS, :].rearrange("(j p) d -> p j d", p=128),
            )
            # halo tile [4, (j, d)]: rows base+128j-4 .. base+128j
            halo_t = halop.tile([4, J * D], bf16, tag="halo", bufs=3)
            if it == 0:
                # j=0 wraps to the last 4 rows of the sequence
                nc.sync.dma_start(
                    out=halo_t[:].rearrange("p (j d) -> p j d", j=J)[:, 0, :],
                    in_=x_d[T - 4:T, :],
                )
                nc.sync.dma_start(
                    out=halo_t[:].rearrange("p (j d) -> p j d", j=J)[:, 1:, :],
                    in_=_subap(x_d[124:128, :], 0, [[128 * D, J - 1], [1, D]]),
                )
            else:
                nc.sync.dma_start(
                    out=halo_t[:].rearrange("p (j d) -> p j d", j=J),
                    in_=_subap(x_d[base - 4:base, :], 0, [[128 * D, J], [1, D]]),
                )
            return x_t, halo_t

        pending_store = None
        cur = emit_loads(0)
        for it in range(n_iter):
            base = it * ROWS
            x_t, halo_t = cur
            # prefetch next iteration's tiles BEFORE this iteration's
            # transpose occupies the sync queue, so the PE never starves
            if it + 1 < n_iter:
                cur = emit_loads(it + 1)

            # ---- d-major copy of x via the DMA xbar transpose engine:
            # xT[dlo, (j*8+dc), t] = x[t, j*1024 + dc*128 + dlo]
            xT_t = xtp.tile([128, J * D], bf16)
            nc.sync.dma_start_transpose(
                out=xT_t[:].rearrange("p (c q) -> p c q", q=128), in_=x_t[:])

            # previous iteration's store goes last on the queue
            if pending_store is not None:
                pending_store()
                pending_store = None

            # ---- stencil u on TensorE; 8 PSUM eighths of 1 block each ----
            u_t = utp.tile([128, J * D], bf16)
            for j in range(J):
                u_ps = psum.tile([128, D], f32, tag="ups", bufs=3)
                for c in range(2):
                    sl_p = slice(c * 512, (c + 1) * 512)
                    sl_x = slice(j * D + c * 512, j * D + (c + 1) * 512)
                    nc.tensor.matmul(u_ps[:, sl_p], lhsT=wm_sb[:],
                                     rhs=x_t[:, sl_x], start=True, stop=False)
                    nc.tensor.matmul(u_ps[:, sl_p], lhsT=ww_sb[:],
                                     rhs=halo_t[:, sl_x], start=False, stop=True)
                # evacuate -> bf16 SBUF (ScalarE), folding the Cayley-diagonal
                # signs: u_t holds uw = w0 * u (negative blades at cols 0..383)
                nc.scalar.activation(
                    out=u_t[:, j * D:j * D + 384], in_=u_ps[:, :384],
                    func=mybir.ActivationFunctionType.Copy, scale=-1.0)
                nc.scalar.activation(
                    out=u_t[:, j * D + 384:(j + 1) * D], in_=u_ps[:, 384:],
                    func=mybir.ActivationFunctionType.Copy)

            # ---- gate matvec on TensorE: lhsT = xT chunk, rhs = gw column --
            gps = psum.tile([128, J], f32, tag="gps", bufs=2)
            for j in range(J):
                for dc in range(8):
                    nc.tensor.matmul(
                        gps[:, j:j + 1],
                        lhsT=xT_t[:, (j * 8 + dc) * 128:(j * 8 + dc + 1) * 128],
                        rhs=gw_sb[:, dc:dc + 1],
                        start=(dc == 0), stop=(dc == 7))
            gate = smallp.tile([128, J], f32, tag="gate")
            nc.scalar.activation(out=gate[:], in_=gps[:],
                                 func=mybir.ActivationFunctionType.Sigmoid,
                                 bias=float(gb), scale=1.0)
            gate_sb = smallp.tile([128, J], f32, tag="gate_sb")
            nc.vector.tensor_scalar_mul(gate_sb[:], gate[:], float(sb))

            # ---- scalar part: sacc[:, j] = sum_d x*uw (signs pre-folded) --
            # fused multiply+reduce STT; 1x mode, but one pass per block
            sacc = smallp.tile([128, J], f32, tag="sacc")
            scr_s = scrp.tile([128, D], bf16, tag="scr_s", bufs=1)
            for j in range(J):
                nc.vector.scalar_tensor_tensor(
                    out=scr_s[:],
                    in0=x_t[:, j * D:(j + 1) * D], scalar=1.0,
                    in1=u_t[:, j * D:(j + 1) * D], op0=mult, op1=mult,
                    accum_out=sacc[:, j:j + 1],
                )

            # ---- wedge pair products (positions: 1->7, 2->8, 4->9, 8->0) --
            wF = wp.tile([128, 6 * J * 64], bf16, tag="wF", bufs=1)
            wR = wp.tile([128, 6 * J * 64], bf16, tag="wR", bufs=1)
            w_t = wp.tile([128, 6 * J * 64], bf16, tag="w", bufs=1)
            jn = [[D, J], [1, 64]]           # (j, n) dims on x/u tiles
            PJ = J * 64                       # pair stride on w tiles

            def prod(dst, pr0, npr, xoff, xstep, uoff, ustep):
                nc.vector.tensor_tensor(
                    out=_subap(dst[:], pr0 * PJ, [[PJ, npr], [64, J], [1, 64]]),
                    in0=_subap(x_t[:], xoff * 64, [[xstep * 64, npr]] + jn),
                    in1=_subap(u_t[:], uoff * 64, [[ustep * 64, npr]] + jn),
                    op=mult,
                )

            # forward x_p * uw_q: pairs (1,2),(1,4) | (2,4) | (1,8),(2,8),(4,8)
            # NOTE uw at position 0 is -u_8, so the pair-3..5 forward products
            # come out negated; those pairs use w = -(F' + R') handled below.
            prod(wF, 0, 2, 7, 0, 8, 1)
            prod(wF, 2, 1, 8, 1, 9, 0)
            prod(wF, 3, 3, 7, 1, 0, 0)
            # reverse x_q * uw_p (positions 7,8,9 are w0-positive: plain u)
            prod(wR, 0, 2, 8, 1, 7, 0)
            prod(wR, 2, 1, 9, 1, 8, 0)
            prod(wR, 3, 3, 0, 0, 7, 1)
            # pairs 0..2: w = F - R ; pairs 3..5: wB = F' + R' = -w
            nc.vector.tensor_tensor(out=w_t[:, :3 * PJ], in0=wF[:, :3 * PJ],
                                    in1=wR[:, :3 * PJ], op=sub_op)
            nc.vector.tensor_tensor(out=w_t[:, 3 * PJ:], in0=wF[:, 3 * PJ:],
                                    in1=wR[:, 3 * PJ:], op=add)

            # ---- scale by sb*gate (per-block TS, 4x), then assemble ----
            gwt = wp.tile([128, 6 * J * 64], bf16, tag="gwt", bufs=1)
            for j in range(J):
                nc.vector.tensor_scalar(
                    out=_subap(gwt[:], j * 64, [[PJ, 6], [1, 64]]),
                    in0=_subap(w_t[:], j * 64, [[PJ, 6], [1, 64]]),
                    scalar1=gate_sb[:, j:j + 1], scalar2=None, op0=mult,
                )
            out_t = outp.tile([128, J * W], bf16)
            # neg bivectors k=3,5,6 at positions 1,2,3 -> cols 0..191
            nc.vector.tensor_tensor(
                out=_subap(out_t[:], 0, [[W, J], [64, 3], [1, 64]]),
                in0=_subap(gwt[:], 0, [[64, J], [PJ, 3], [1, 64]]),
                in1=_subap(x_t[:], 64, [[D, J], [64, 3], [1, 64]]),
                op=add,
            )
            # pos bivectors k=9,10,12 at 10,11,12 -> cols 192..383 (w = -wB)
            nc.vector.tensor_tensor(
                out=_subap(out_t[:], 192, [[W, J], [64, 3], [1, 64]]),
                in0=_subap(x_t[:], 640, [[D, J], [64, 3], [1, 64]]),
                in1=_subap(gwt[:], 3 * PJ, [[64, J], [PJ, 3], [1, 64]]),
                op=sub_op,
            )

            # ---- scalar column: out[:, 384] = x0 + ss*gate*sacc ----
            gs = smallp.tile([128, J], f32, tag="gs")
            nc.vector.tensor_tensor(out=gs[:], in0=gate[:], in1=sacc[:], op=mult)
            nc.vector.scalar_tensor_tensor(
                out=_subap(out_t[:], 384, [[W, J]]),
                in0=gs[:], scalar=float(ss),
                in1=_subap(x_t[:], 384, [[D, J]]),   # blade 0 at position 6
                op0=mult, op1=add,
            )

            # ---- store compact tile (deferred; see top of loop) ----
            def make_store(base=base, out_t=out_t):
                def store():
                    nc.sync.dma_start(
                        out=out_d[base:base + ROWS, :].rearrange(
                            "(j p) w -> p j w", p=128),
                        in_=out_t[:].rearrange("p (j w) -> p j w", j=J),
                    )
                return store
            pending_store = make_store()

        pending_store()

    nc.compile()
    return nc


def _get_program(T, D, ss, sb, gb):
    key = (T, D, round(ss, 9), round(sb, 9), round(gb, 9))
    if key not in _PROG_CACHE:
        _PROG_CACHE[key] = build_program(T, D, ss, sb, gb)
    return _PROG_CACHE[key]


def _permute_cols(a2d, D):
    """[.., D] f32 -> blade-major bf16: position p*64+n <- blade ORD[p], chunk n."""
    n = D // ALG
    r = a2d.reshape(a2d.shape[:-1] + (n, ALG))
    r = r[..., ORD]                      # [..., n, 16] with blades reordered
    r = np.swapaxes(r, -1, -2)           # [..., 16, n]
    return np.ascontiguousarray(r.reshape(a2d.shape[:-1] + (D,)).astype(BF16))


def kernel(x, gate_w, gate_b, scalar_weight, bivector_weight):
    x = np.asarray(x, np.float32)
    B, T, D = x.shape
    assert B == 8 and D == 1024

    ss = _sigmoid_f32(np.asarray(scalar_weight).reshape(-1)[0])
    sb = _sigmoid_f32(np.asarray(bivector_weight).reshape(-1)[0])
    gb = float(np.asarray(gate_b).reshape(-1)[0])

    nc = _get_program(T, D, ss, sb, gb)

    from concourse.bass_utils import run_bass_kernel_spmd

    gw = np.asarray(gate_w, np.float32).reshape(D)
    gw_bm = _permute_cols(gw[None, :], D)[0].astype(np.float32)
    # d-major gw for the TensorE matvec: gwcol[dlo, dc] = gw_bm[dc*128+dlo]
    gwcol = np.ascontiguousarray(gw_bm.reshape(8, 128).T.astype(BF16))
    wm, ww = _stencil_weights()
    in_maps = []
    for c in range(B):
        in_maps.append({
            "x": _permute_cols(x[c], D),
            "gwcol": gwcol,
            "wmain": wm,
            "wwrap": ww,
        })
    res = run_bass_kernel_spmd(nc, in_maps, list(range(B)), trace=TRACE)
    global LAST_RESULT
    LAST_RESULT = res

    # host-side scatter: only 385 of 1024 positions differ from x
    out = x.copy()
    kcols = np.array([16 * n + (p ^ q) for (p, q) in PAIRS for n in range(64)])
    for c in range(B):
        o = np.asarray(res.results[c]["out"], dtype=np.float32)  # [T, 392]
        out[c][:, kcols] = o[:, :384]
        out[c][:, 0] = o[:, 384]
    return out


# revision 43
# speedup vs baseline: 1.0311x; 1.0055x over previous
"""Trainium2 Bass kernel for nn_CliffordInteractionExpert (v2: bf16 blade-major).

Math (CliffordAlgebra p=3,q=1: ALG=16 blades, D=1024 = 64 chunks of 16):
  All three shifts are linear, so they collapse into one stencil:
      u = 3x - x<<1 - x<<2 - x<<4   (roll along T, wraparound)
  out = x + gate * [ sb*(x_p u_q - x_q u_p) at bivector blades,
                     ss*sum_d w0*x*u       at d=0 ]
  gate = sigmoid(x @ gate_w + gate_b)

Key layout decisions (vs v1):
  - Everything in bf16 on device (tolerance 2e-2; measured err stays ~4e-3).
  - Host permutes D into "blade-major" order: position p*64+n holds blade
    ORD[p] of chunk n, with ORD = [8,3,5,6,7,15, 0,1,2,4, 9,10,12, 11,13,14].
    * w0-negative blades occupy positions 0..5  -> the Cayley-diagonal
      reduction is two contiguous fused multiply-reduce ops (neg / pos).
    * vector blades 1,2,4 at positions 7,8,9 and 8 at position 0 -> the six
      wedge pair-products group into 6 strided step-1 ops (DVE 2x mode).
    * bivector outputs 3,5,6 at positions 1,2,3 and 9,10,12 at 10,11,12 ->
      gated output assembly is two step-1 ops per 128-row block.
  - Output is compact [T, 392] (384 bivector cols + 1 scalar col + pad):
    only 385 of 1024 positions differ from x; host scatters them into a
    copy of x. Cuts store traffic 5x.
  - Stencil u on TensorE as banded-matrix matmuls (bf16, half the cycles of
    fp32); halo rows for block j>0 come from partitions 124..127 of block
    j-1 in-tile; only j=0 reads a 4-row halo from DRAM.
  - PSUM is split in 4 quarters (2 blocks each) with bufs=2 so evacuation
    (ScalarE copy -> bf16 SBUF) overlaps the next quarter's matmuls.
"""

import numpy as np
import ml_dtypes

BF16 = ml_dtypes.bfloat16
ALG = 16
SHIFTS = (1, 2, 4)
# blade at position p of each 64-wide block (see module docstring)
ORD = [8, 3, 5, 6, 7, 15, 0, 1, 2, 4, 9, 10, 12, 11, 13, 14]
# wedge pairs (p_blade, q_blade) -> bivector k = p^q; order chosen so
# k-positions are [1,2,3, 10,11,12] in ORD space (step-1 groups)
PAIRS = [(1, 2), (1, 4), (2, 4), (1, 8), (2, 8), (4, 8)]  # k = 3,5,6, 9,10,12
POS = {b: p for p, b in enumerate(ORD)}

_PROG_CACHE: dict = {}
TRACE = False
LAST_RESULT = None


def _sigmoid_f32(v: float) -> float:
    return float(1.0 / (1.0 + np.exp(-np.float32(v), dtype=np.float32)))


def _stencil_weights():
    """u = 3x - x[t-1] - x[t-2] - x[t-4] as lhsT banded matrices.

    wm[s, t]: weight of in-block row s for output row t (128x128).
    ww[h, t]: weight of halo row h (the 4 rows preceding the block).
    """
    wm = np.zeros((128, 128), np.float32)
    ww = np.zeros((4, 128), np.float32)
    for t in range(128):
        wm[t, t] = 3.0
        for k in SHIFTS:
            if t - k >= 0:
                wm[t - k, t] -= 1.0
            else:
                ww[4 + t - k, t] -= 1.0
    return wm.astype(BF16), ww.astype(BF16)


def _subap(base, elem_off, dims):
    """AP at base's tensor with extra element offset and explicit free dims."""
    import concourse.bass as bass

    return bass.AP(tensor=base.tensor, offset=base.offset + elem_off,
                   ap=[list(base.ap[0])] + [list(d) for d in dims])


def build_program(T: int, D: int, ss: float, sb: float, gb: float):
    from contextlib import ExitStack

    import concourse.bacc as bacc
    import concourse.mybir as mybir
    from concourse.tile import TileContext

    bf16 = mybir.dt.bfloat16
    f32 = mybir.dt.float32
    J = 8                  # 128-row blocks per iteration
    ROWS = 128 * J         # 1024
    W = 392                # compact output row width
    assert T % ROWS == 0 and D == 1024
    n_iter = T // ROWS

    nc = bacc.Bacc("TRN2", target_bir_lowering=False, debug=False)
    x_d = nc.dram_tensor("x", [T, D], bf16, kind="ExternalInput")
    gw_d = nc.dram_tensor("gwcol", [128, D // 128], bf16, kind="ExternalInput")
    wm_d = nc.dram_tensor("wmain", [128, 128], bf16, kind="ExternalInput")
    ww_d = nc.dram_tensor("wwrap", [4, 128], bf16, kind="ExternalInput")
    out_d = nc.dram_tensor("out", [T, W], bf16, kind="ExternalOutput")

    mult = mybir.AluOpType.mult
    add = mybir.AluOpType.add
    sub_op = mybir.AluOpType.subtract

    with TileContext(nc) as tc, ExitStack() as ctx:
        consts = ctx.enter_context(tc.tile_pool(name="consts", bufs=1))
        xp = ctx.enter_context(tc.tile_pool(name="xp", bufs=2))
        xtp = ctx.enter_context(tc.tile_pool(name="xtp", bufs=2))
        utp = ctx.enter_context(tc.tile_pool(name="utp", bufs=2))
        wp = ctx.enter_context(tc.tile_pool(name="wp", bufs=2))
        outp = ctx.enter_context(tc.tile_pool(name="outp", bufs=2))
        scrp = ctx.enter_context(tc.tile_pool(name="scrp", bufs=2))
        smallp = ctx.enter_context(tc.tile_pool(name="smallp", bufs=4))
        halop = ctx.enter_context(tc.tile_pool(name="halop", bufs=2))
        psum = ctx.enter_context(tc.tile_pool(name="psum", bufs=2, space="PSUM"))

        gw_sb = consts.tile([128, D // 128], bf16)
        nc.sync.dma_start(out=gw_sb[:], in_=gw_d[:])
        wm_sb = consts.tile([128, 128], bf16)
        nc.sync.dma_start(out=wm_sb[:], in_=wm_d[:])
        ww_sb = consts.tile([4, 128], bf16)
        nc.sync.dma_start(out=ww_sb[:], in_=ww_d[:])
        id1 = consts.tile([1, 1], bf16)
        nc.vector.memset(id1[:], 1.0)

        def emit_loads(it):
            base = it * ROWS
            # x tile [128, (j, d)]: row t = base + 128j + p
            x_t = xp.tile([128, J * D], bf16, tag="x", bufs=3)
            nc.sync.dma_start(
                out=x_t[:].rearrange("p (j d) -> p j d", j=J),
                in_=x_d[base:base + ROWS, :].rearrange("(j p) d -> p j d", p=128),
            )
            # halo tile [4, (j, d)]: rows base+128j-4 .. base+128j
            halo_t = halop.tile([4, J * D], bf16, tag="halo", bufs=2)
            if it == 0:
                # j=0 wraps to the last 4 rows of the sequence
                nc.sync.dma_start(
                    out=halo_t[:].rearrange("p (j d) -> p j d", j=J)[:, 0, :],
                    in_=x_d[T - 4:T, :],
                )
                nc.sync.dma_start(
                    out=halo_t[:].rearrange("p (j d) -> p j d", j=J)[:, 1:, :],
                    in_=_subap(x_d[124:128, :], 0, [[128 * D, J - 1], [1, D]]),
                )
            else:
                nc.sync.dma_start(
                    out=halo_t[:].rearrange("p (j d) -> p j d", j=J),
                    in_=_subap(x_d[base - 4:base, :], 0, [[128 * D, J], [1, D]]),
                )
            return x_t, halo_t

        pending_store = None
        cur = emit_loads(0)
        for it in range(n_iter):
            base = it * ROWS
            x_t, halo_t = cur
            # prefetch next iteration's tiles BEFORE this iteration's
            # transpose occupies the sync queue, so the PE never starves
            if it + 1 < n_iter:
                cur = emit_loads(it + 1)

            # ---- d-major copy of x via the DMA xbar transpose engine:
            # xT[dlo, (j*8+dc), t] = x[t, j*1024 + dc*128 + dlo]
            xT_t = xtp.tile([128, J * D], bf16, tag="xT", bufs=3)
            nc.sync.dma_start_transpose(
                out=xT_t[:].rearrange("p (c q) -> p c q", q=128), in_=x_t[:])

            # previous iteration's store goes last on the queue
            if pending_store is not None:
                pending_store()
                pending_store = None

            # ---- gate matvec on TensorE: lhsT = xT chunk, rhs = gw column --
            # cheap (~1.7us, fully pipelined); emitted BEFORE the stencil for
            # it>0 so sigma/gate_sb are ready early and the xT buffer frees
            # early (iteration 0 keeps stencil first: xT waits on the first
            # transpose, which would stall the whole PE queue at startup)
            def emit_gate_mms():
                for j in range(J):
                    for dc in range(8):
                        nc.tensor.matmul(
                            gps[:, j:j + 1],
                            lhsT=xT_t[:, (j * 8 + dc) * 128:
                                      (j * 8 + dc + 1) * 128],
                            rhs=gw_sb[:, dc:dc + 1],
                            start=(dc == 0), stop=(dc == 7))

            gps = psum.tile([128, J], f32, tag="gps", bufs=2)
            if it > 0:
                emit_gate_mms()

            # ---- stencil u on TensorE; 8 PSUM eighths of 1 block each ----
            u_t = utp.tile([128, J * D], bf16)
            for j in range(J):
                u_ps = psum.tile([128, D], f32, tag="ups", bufs=3)
                for c in range(2):
                    sl_p = slice(c * 512, (c + 1) * 512)
                    sl_x = slice(j * D + c * 512, j * D + (c + 1) * 512)
                    nc.tensor.matmul(u_ps[:, sl_p], lhsT=wm_sb[:],
                                     rhs=x_t[:, sl_x], start=True, stop=False)
                    nc.tensor.matmul(u_ps[:, sl_p], lhsT=ww_sb[:],
                                     rhs=halo_t[:, sl_x], start=False, stop=True)
                # evacuate -> bf16 SBUF (ScalarE), folding the Cayley-diagonal
                # signs: u_t holds uw = w0 * u (negative blades at cols 0..383)
                nc.scalar.activation(
                    out=u_t[:, j * D:j * D + 384], in_=u_ps[:, :384],
                    func=mybir.ActivationFunctionType.Copy, scale=-1.0)
                nc.scalar.activation(
                    out=u_t[:, j * D + 384:(j + 1) * D], in_=u_ps[:, 384:],
                    func=mybir.ActivationFunctionType.Copy)

            if it == 0:
                emit_gate_mms()
            gate = smallp.tile([128, J], f32, tag="gate")
            nc.scalar.activation(out=gate[:], in_=gps[:],
                                 func=mybir.ActivationFunctionType.Sigmoid,
                                 bias=float(gb), scale=1.0)
            gate_sb = smallp.tile([128, J], f32, tag="gate_sb")
            nc.vector.tensor_scalar_mul(gate_sb[:], gate[:], float(sb))

            # ---- scalar part: sacc[:, j] = sum_d x*uw (signs pre-folded) --
            # fused multiply+reduce STT; 1x mode, but one pass per block
            sacc = smallp.tile([128, J], f32, tag="sacc")
            scr_s = scrp.tile([128, D], bf16, tag="scr_s", bufs=1)
            for j in range(J):
                nc.vector.scalar_tensor_tensor(
                    out=scr_s[:],
                    in0=x_t[:, j * D:(j + 1) * D], scalar=1.0,
                    in1=u_t[:, j * D:(j + 1) * D], op0=mult, op1=mult,
                    accum_out=sacc[:, j:j + 1],
                )

            # ---- wedge pair products (positions: 1->7, 2->8, 4->9, 8->0) --
            wF = wp.tile([128, 6 * J * 64], bf16, tag="wF", bufs=1)
            wR = wp.tile([128, 6 * J * 64], bf16, tag="wR", bufs=1)
            w_t = wp.tile([128, 6 * J * 64], bf16, tag="w", bufs=1)
            jn = [[D, J], [1, 64]]           # (j, n) dims on x/u tiles
            PJ = J * 64                       # pair stride on w tiles

            def prod(dst, pr0, npr, xoff, xstep, uoff, ustep):
                nc.vector.tensor_tensor(
                    out=_subap(dst[:], pr0 * PJ, [[PJ, npr], [64, J], [1, 64]]),
                    in0=_subap(x_t[:], xoff * 64, [[xstep * 64, npr]] + jn),
                    in1=_subap(u_t[:], uoff * 64, [[ustep * 64, npr]] + jn),
                    op=mult,
                )

            # forward x_p * uw_q: pairs (1,2),(1,4) | (2,4) | (1,8),(2,8),(4,8)
            # NOTE uw at position 0 is -u_8, so the pair-3..5 forward products
            # come out negated; those pairs use w = -(F' + R') handled below.
            prod(wF, 0, 2, 7, 0, 8, 1)
            prod(wF, 2, 1, 8, 1, 9, 0)
            prod(wF, 3, 3, 7, 1, 0, 0)
            # reverse x_q * uw_p (positions 7,8,9 are w0-positive: plain u)
            prod(wR, 0, 2, 8, 1, 7, 0)
            prod(wR, 2, 1, 9, 1, 8, 0)
            prod(wR, 3, 3, 0, 0, 7, 1)
            # pairs 0..2: w = F - R ; pairs 3..5: wB = F' + R' = -w
            nc.vector.tensor_tensor(out=w_t[:, :3 * PJ], in0=wF[:, :3 * PJ],
                                    in1=wR[:, :3 * PJ], op=sub_op)
            nc.vector.tensor_tensor(out=w_t[:, 3 * PJ:], in0=wF[:, 3 * PJ:],
                                    in1=wR[:, 3 * PJ:], op=add)

            # ---- scale by sb*gate (per-block TS, 4x), then assemble ----
            gwt = wp.tile([128, 6 * J * 64], bf16, tag="gwt", bufs=1)
            for j in range(J):
                nc.vector.tensor_scalar(
                    out=_subap(gwt[:], j * 64, [[PJ, 6], [1, 64]]),
                    in0=_subap(w_t[:], j * 64, [[PJ, 6], [1, 64]]),
                    scalar1=gate_sb[:, j:j + 1], scalar2=None, op0=mult,
                )
            out_t = outp.tile([128, J * W], bf16)
            # neg bivectors k=3,5,6 at positions 1,2,3 -> cols 0..191
            nc.vector.tensor_tensor(
                out=_subap(out_t[:], 0, [[W, J], [64, 3], [1, 64]]),
                in0=_subap(gwt[:], 0, [[64, J], [PJ, 3], [1, 64]]),
                in1=_subap(x_t[:], 64, [[D, J], [64, 3], [1, 64]]),
                op=add,
            )
            # pos bivectors k=9,10,12 at 10,11,12 -> cols 192..383 (w = -wB)
            nc.vector.tensor_tensor(
                out=_subap(out_t[:], 192, [[W, J], [64, 3], [1, 64]]),
                in0=_subap(x_t[:], 640, [[D, J], [64, 3], [1, 64]]),
                in1=_subap(gwt[:], 3 * PJ, [[64, J], [PJ, 3], [1, 64]]),
                op=sub_op,
            )

            # ---- scalar column: out[:, 384] = x0 + ss*gate*sacc ----
            gs = smallp.tile([128, J], f32, tag="gs")
            nc.vector.tensor_tensor(out=gs[:], in0=gate[:], in1=sacc[:], op=mult)
            nc.vector.scalar_tensor_tensor(
                out=_subap(out_t[:], 384, [[W, J]]),
                in0=gs[:], scalar=float(ss),
                in1=_subap(x_t[:], 384, [[D, J]]),   # blade 0 at position 6
                op0=mult, op1=add,
            )

            # ---- store compact tile (deferred; see top of loop) ----
            def make_store(base=base, out_t=out_t):
                def store():
                    nc.sync.dma_start(
                        out=out_d[base:base + ROWS, :].rearrange(
                            "(j p) w -> p j w", p=128),
                        in_=out_t[:].rearrange("p (j w) -> p j w", j=J),
                    )
                return store
            pending_store = make_store()

        pending_store()

    nc.compile()
    return nc


def _get_program(T, D, ss, sb, gb):
    key = (T, D, round(ss, 9), round(sb, 9), round(gb, 9))
    if key not in _PROG_CACHE:
        _PROG_CACHE[key] = build_program(T, D, ss, sb, gb)
    return _PROG_CACHE[key]


def _permute_cols(a2d, D):
    """[.., D] f32 -> blade-major bf16: position p*64+n <- blade ORD[p], chunk n."""
    n = D // ALG
    r = a2d.reshape(a2d.shape[:-1] + (n, ALG))
    r = r[..., ORD]                      # [..., n, 16] with blades reordered
    r = np.swapaxes(r, -1, -2)           # [..., 16, n]
    return np.ascontiguousarray(r.reshape(a2d.shape[:-1] + (D,)).astype(BF16))


def kernel(x, gate_w, gate_b, scalar_weight, bivector_weight):
    x = np.asarray(x, np.float32)
    B, T, D = x.shape
    assert B == 8 and D == 1024

    ss = _sigmoid_f32(np.asarray(scalar_weight).reshape(-1)[0])
    sb = _sigmoid_f32(np.asarray(bivector_weight).reshape(-1)[0])
    gb = float(np.asarray(gate_b).reshape(-1)[0])

    nc = _get_program(T, D, ss, sb, gb)

    from concourse.bass_utils import run_bass_kernel_spmd

    gw = np.asarray(gate_w, np.float32).reshape(D)
    gw_bm = _permute_cols(gw[None, :], D)[0].astype(np.float32)
    # d-major gw for the TensorE matvec: gwcol[dlo, dc] = gw_bm[dc*128+dlo]
    gwcol = np.ascontiguousarray(gw_bm.reshape(8, 128).T.astype(BF16))
    wm, ww = _stencil_weights()
    in_maps = []
    for c in range(B):
        in_maps.append({
            "x": _permute_cols(x[c], D),
            "gwcol": gwcol,
            "wmain": wm,
            "wwrap": ww,
        })
    res = run_bass_kernel_spmd(nc, in_maps, list(range(B)), trace=TRACE)
    global LAST_RESULT
    LAST_RESULT = res

    # host-side scatter: only 385 of 1024 positions differ from x
    out = x.copy()
    kcols = np.array([16 * n + (p ^ q) for (p, q) in PAIRS for n in range(64)])
    for c in range(B):
        o = np.asarray(res.results[c]["out"], dtype=np.float32)  # [T, 392]
        out[c][:, kcols] = o[:, :384]
        out[c][:, 0] = o[:, 384]
    return out
